# revision 13
# baseline (speedup 1.0000x reference)
"""Trainium2 Bass kernel for nn_AttentionUnit (attention pooling).

reference math:
    q = query @ Wq.T + bq;  k = keys @ Wk.T + bk
    score[b,l] = <k[b,l], q[b]>;  attn = softmax(score, axis=1)
    wsum[b] = sum_l attn[b,l] * keys[b,l];  return (wsum, attn)

Algebra: score[b,l] = <keys[b,l,:], qk[b,:]> + c[b], qk = (query@Wq.T+bq)@Wk;
c[b] is constant over l so it cancels in the softmax -> bk never matters and
the big keys projection disappears.  Memory-bound on reading keys once.

Per-batch-row work is mapped to the tensor engine as M=1 matmuls with the
keys tile as the N-moving operand (weights = a single qk / exp column, so
LDWEIGHTS is 1 column instead of 100+), packed 4-wide into the PE's 32-col
groups via tile_position so four rows compute concurrently:
  score_b = qk_b.T @ keysT_b      (K=i=128, N=L=200)  -> psum row 32g
  uws_b   = expT_b.T @ keysN_b    (K=l=100 x2 acc, N=D=128) -> psum row 32g
Rows land on psum partitions {0,32,64,96} x 8 round-offsets; ACT/DVE copies
with partition-stride-32 APs compact them to [b, .] tiles.  Softmax runs in
[b, l] layout: DVE reduce_max, ACT exp with bias=-max + accumulated sums,
DVE reciprocal/normalize.  exp is transposed (PE) and cast to fp16 for the
wsum pass; keysN is fp16 (keys are N(0,1): rel err ~1e-4, well in range).
"""

import sys

sys.path.insert(0, "/opt/trn_rl_repo")

import numpy as np

D = 128
L = 200
LH = L // 2  # 100
B_FULL = 8192
N_CORES = 8
B_CORE = B_FULL // N_CORES  # 1024

_NC_CACHE = {}


def build_nc(b_core=B_CORE, tile_b=32, kt_fp16=False):
    from contextlib import ExitStack

    import concourse.masks as masks
    import concourse.mybir as mybir
    import concourse.tile as tile
    from concourse.bacc import Bacc

    f32 = mybir.dt.float32
    f16 = mybir.dt.float16
    ktd = f16 if kt_fp16 else f32
    AF = mybir.ActivationFunctionType
    OP = mybir.AluOpType

    assert b_core % tile_b == 0 and tile_b % 4 == 0
    n_tiles = b_core // tile_b
    R = tile_b // 4  # rounds of 4 concurrent col-groups
    assert b_core % 128 == 0

    nc = Bacc()

    # i-major / l-major global layouts so each DMA descriptor is one long
    # contiguous run per partition
    keysT_d = nc.dram_tensor("keysT", [D, b_core, L], ktd, kind="ExternalInput")
    keysN_d = nc.dram_tensor("keysN", [LH, b_core, 2 * D], f16, kind="ExternalInput")
    queryT_d = nc.dram_tensor("queryT", [D, b_core], f32, kind="ExternalInput")
    wqT_d = nc.dram_tensor("wqT", [D, D], f32, kind="ExternalInput")
    wk_d = nc.dram_tensor("wk", [D, D], f32, kind="ExternalInput")
    bq_d = nc.dram_tensor("bq", [D, 1], f32, kind="ExternalInput")
    wsum_d = nc.dram_tensor("wsum", [b_core, D], f32, kind="ExternalOutput")
    attn_d = nc.dram_tensor("attn", [b_core, L], f32, kind="ExternalOutput")

    with tile.TileContext(nc) as tc, ExitStack() as ctx:
        const = ctx.enter_context(tc.tile_pool(name="const", bufs=1))
        identity = const.tile([128, 128], f32)
        masks.make_identity(nc, identity[:])
        queryT_s = const.tile([D, b_core], f32)
        nc.sync.dma_start(queryT_s[:], queryT_d[:])
        qkT_all = const.tile([D, b_core], ktd)

        kt_p = ctx.enter_context(tc.tile_pool(name="kt", bufs=2))
        kn_p = ctx.enter_context(tc.tile_pool(name="kn", bufs=3))
        sm_p = ctx.enter_context(tc.tile_pool(name="sm", bufs=3))
        out_p = ctx.enter_context(tc.tile_pool(name="outs", bufs=2))
        # PSUM: 4 + 2 + 2 = 8 banks
        ps_sc = ctx.enter_context(tc.tile_pool(name="ps_sc", bufs=1, space="PSUM"))
        ps_uw = ctx.enter_context(tc.tile_pool(name="ps_uw", bufs=1, space="PSUM"))
        ps_tr = ctx.enter_context(tc.tile_pool(name="ps_tr", bufs=2, space="PSUM"))
        # persistent psum blocks (memset once so the never-written garbage
        # rows between the M=1 outputs are initialized)
        sc_ps = ps_sc.tile([128, R * 256], f32)
        nc.vector.memset(sc_ps[:], 0.0)
        uws_ps = ps_uw.tile([128, R * 128], f32)
        nc.vector.memset(uws_ps[:], 0.0)

        # ---------------- pre-pass: qkT = Wk-contract(WqT.T @ queryT + bq) --
        with tc.tile_pool(name="pre_sb", bufs=2) as pre_sb:
            wqT_s = pre_sb.tile([D, D], f32, tag="w")
            nc.sync.dma_start(wqT_s[:], wqT_d[:])
            wk_s = pre_sb.tile([D, D], f32, tag="w")
            nc.sync.dma_start(wk_s[:], wk_d[:])
            bq_s = pre_sb.tile([D, 1], f32, tag="b")
            nc.sync.dma_start(bq_s[:], bq_d[:])
            for t8 in range(b_core // 128):
                sl = slice(t8 * 128, (t8 + 1) * 128)
                qT_ps = ps_tr.tile([128, 128], f32, tag="tr")
                nc.tensor.matmul(qT_ps[:], wqT_s[:], queryT_s[:, sl])
                qT_sb = pre_sb.tile([128, 128], f32, tag="q")
                nc.scalar.activation(qT_sb[:], qT_ps[:], AF.Identity, bias=bq_s[:, 0:1])
                qk_ps = ps_tr.tile([128, 128], f32, tag="tr")
                nc.tensor.matmul(qk_ps[:], wk_s[:], qT_sb[:])
                nc.scalar.activation(qkT_all[:, sl], qk_ps[:], AF.Copy, bias=0.0)

        # ---------------- main pipeline -------------------------------------
        # working layout: local row j = 4*r + g lives at partition g, round r
        state = {}

        def emit_load_score(t):
            t0 = t * tile_b
            kT = kt_p.tile([D, tile_b, L], ktd, tag="kt")
            nc.sync.dma_start(kT[:], keysT_d[:, t0 : t0 + tile_b, :])
            kN = kn_p.tile([LH, tile_b, 2 * D], f16, tag="kn")
            nc.sync.dma_start(kN[:], keysN_d[:, t0 : t0 + tile_b, :])
            # score rows: group g -> psum partition 32g, round r -> 256-fl offset
            for r in range(R):
                for g in range(4):
                    j = 4 * r + g
                    nc.tensor.matmul(
                        sc_ps[32 * g : 32 * g + 1, r * 256 : r * 256 + L],
                        qkT_all[:, t0 + j : t0 + j + 1],
                        kT[:, j, :],
                        tile_position=(0, 32 * g),
                    )
            state[t] = kN

        def emit_softmax(t):
            kN = state[t]
            t0 = t * tile_b
            # full-width softmax over the sparse psum rows (only partitions
            # {0,32,64,96} carry data; other lanes compute garbage for free)
            scv = sc_ps[:].rearrange("p (r x) -> p r x", x=256)[:, :, 0:L]
            m128 = sm_p.tile([128, R], f32, tag="m")
            nc.vector.tensor_reduce(m128[:], scv, axis=mybir.AxisListType.X, op=OP.max)
            m128b = m128[:].unsqueeze(2).broadcast_to([128, R, L])
            sh128 = sm_p.tile([128, R, L], f32, tag="sh")
            nc.vector.tensor_tensor(sh128[:], scv, m128b, op=OP.subtract)
            exp128 = sm_p.tile([128, R, L], f32, tag="e")
            nc.scalar.activation(exp128[:], sh128[:], AF.Exp, bias=0.0)
            s128 = sm_p.tile([128, R], f32, tag="s")
            nc.vector.tensor_reduce(s128[:], exp128[:], axis=mybir.AxisListType.X, op=OP.add)
            r128 = sm_p.tile([128, R], f32, tag="r")
            nc.vector.reciprocal(r128[:], s128[:])
            attn128 = sm_p.tile([128, R, L], f32, tag="at")
            r128b = r128[:].unsqueeze(2).broadcast_to([128, R, L])
            nc.vector.tensor_tensor(attn128[:], exp128[:], r128b, op=OP.mult)
            nc.scalar.dma_start(
                attn_d[t0 : t0 + tile_b, :].rearrange("(r g) l -> g r l", g=4),
                attn128[0:128:32, :, :],
            )
            # compact the valid exp rows via DMA, then transpose+cast to fp16
            exp4 = sm_p.tile([4, R, L], f32, tag="e4")
            nc.scalar.dma_start(exp4[:], exp128[0:128:32, :, :])
            etA = sm_p.tile([LH, tile_b], f16, tag="et16")
            etB = sm_p.tile([LH, tile_b], f16, tag="et16")
            for r in range(R):
                trA = ps_tr.tile([LH, 4], f32, tag="tr")
                nc.tensor.transpose(trA[:], exp4[:, r, 0:LH], identity[0:4, 0:4])
                nc.scalar.activation(etA[:, 4 * r : 4 * r + 4], trA[:], AF.Copy, bias=0.0)
                trB = ps_tr.tile([LH, 4], f32, tag="tr")
                nc.tensor.transpose(trB[:], exp4[:, r, LH:L], identity[0:4, 0:4])
                nc.scalar.activation(etB[:, 4 * r : 4 * r + 4], trB[:], AF.Copy, bias=0.0)
            state[t] = (kN, etA, etB, r128)

        def emit_back(t):
            t0 = t * tile_b
            kN, etA, etB, r128 = state.pop(t)
            for r in range(R):
                for g in range(4):
                    j = 4 * r + g
                    out = uws_ps[32 * g : 32 * g + 1, r * 128 : (r + 1) * 128]
                    nc.tensor.matmul(
                        out, etA[:, j : j + 1], kN[:, j, 0:D],
                        start=True, stop=False, tile_position=(0, 32 * g),
                    )
                    nc.tensor.matmul(
                        out, etB[:, j : j + 1], kN[:, j, D : 2 * D],
                        start=False, stop=True, tile_position=(0, 32 * g),
                    )
            ws128 = out_p.tile([128, R, D], f32, tag="ws")
            uwv = uws_ps[:].rearrange("p (r x) -> p r x", x=D)
            r128b = r128[:].unsqueeze(2).broadcast_to([128, R, D])
            nc.vector.tensor_tensor(ws128[:], uwv, r128b, op=OP.mult)
            nc.scalar.dma_start(
                wsum_d[t0 : t0 + tile_b, :].rearrange("(r g) i -> g r i", g=4),
                ws128[0:128:32, :, :],
            )

        for t in range(n_tiles):
            emit_load_score(t)
            if t > 0:
                emit_back(t - 1)
            emit_softmax(t)
        emit_back(n_tiles - 1)

    nc.finalize()
    return nc


def get_nc(**kw):
    key = tuple(sorted(kw.items()))
    if key not in _NC_CACHE:
        _NC_CACHE[key] = build_nc(**kw)
    return _NC_CACHE[key]


def _prep_in_maps(query, keys, Wq, bq, Wk, b_core=B_CORE, kt_fp16=False):
    query = np.asarray(query, dtype=np.float32)
    keys = np.asarray(keys, dtype=np.float32)
    B = query.shape[0]
    ktd = np.float16 if kt_fp16 else np.float32
    keysT = keys.transpose(2, 0, 1).astype(ktd)  # [D, B, L] i-major
    # pair rows l and l+100 (512B contiguous fp16 rows), l-major globally
    keysN = (
        keys.reshape(B, 2, LH, D).transpose(2, 0, 1, 3).reshape(LH, B, 2 * D)
    ).astype(np.float16)
    wqT = np.ascontiguousarray(np.asarray(Wq, dtype=np.float32).T)
    wk = np.ascontiguousarray(np.asarray(Wk, dtype=np.float32))
    bqc = np.ascontiguousarray(np.asarray(bq, dtype=np.float32).reshape(D, 1))
    queryT = np.ascontiguousarray(query.T)
    in_maps = []
    for c in range(B // b_core):
        s = slice(c * b_core, (c + 1) * b_core)
        in_maps.append(
            {
                "keysT": np.ascontiguousarray(keysT[:, s, :]),
                "keysN": np.ascontiguousarray(keysN[:, s, :]),
                "queryT": np.ascontiguousarray(queryT[:, s]),
                "wqT": wqT,
                "wk": wk,
                "bq": bqc,
            }
        )
    return in_maps


def kernel(query, keys, Wq, bq, Wk, bk):
    from concourse.bass_utils import run_bass_kernel_spmd

    nc = get_nc()
    in_maps = _prep_in_maps(query, keys, Wq, bq, Wk)
    res = run_bass_kernel_spmd(nc, in_maps, list(range(N_CORES))).results
    wsum = np.concatenate([r["wsum"] for r in res], axis=0)
    attn = np.concatenate([r["attn"] for r in res], axis=0)
    return (wsum, attn)


# ----------------------------------------------------------------------------
def _np_reference(query, keys, Wq, bq, Wk, bk):
    q = query @ Wq.T + bq
    k = keys @ Wk.T + bk
    score = np.einsum("bld,bd->bl", k, q)
    m = score.max(axis=1, keepdims=True)
    e = np.exp(score - m)
    attn = e / e.sum(axis=1, keepdims=True)
    ws = np.einsum("bl,bld->bd", attn, keys)
    return ws, attn


def _make_inputs(b, seed=0):
    rng = np.random.default_rng(seed)
    s = 1.0 / np.sqrt(D)
    return dict(
        query=rng.standard_normal((b, D), dtype=np.float32),
        keys=rng.standard_normal((b, L, D), dtype=np.float32),
        Wq=rng.standard_normal((D, D), dtype=np.float32) * s,
        bq=rng.standard_normal((D,), dtype=np.float32) * s,
        Wk=rng.standard_normal((D, D), dtype=np.float32) * s,
        bk=rng.standard_normal((D,), dtype=np.float32) * s,
    )


def _selftest_sim(b_core=128, tile_b=32, kt_fp16=False):
    from concourse.bass_interp import CoreSim

    ins = _make_inputs(b_core)
    nc = build_nc(b_core=b_core, tile_b=tile_b, kt_fp16=kt_fp16)
    in_map = _prep_in_maps(
        ins["query"], ins["keys"], ins["Wq"], ins["bq"], ins["Wk"],
        b_core=b_core, kt_fp16=kt_fp16,
    )[0]
    sim = CoreSim(nc)
    for name, arr in in_map.items():
        sim.tensor(name)[:] = arr
    sim.simulate()
    wsum = np.array(sim.tensor("wsum"))
    attn = np.array(sim.tensor("attn"))
    ws_ref, attn_ref = _np_reference(
        ins["query"], ins["keys"], ins["Wq"], ins["bq"], ins["Wk"], ins["bk"]
    )
    for name, got, ref, tol in (
        ("wsum", wsum, ws_ref, 2e-3),
        ("attn", attn, attn_ref, 2e-2 if kt_fp16 else 2e-4),
    ):
        err = np.abs(got - ref).max() / (np.abs(ref).max() + 1e-30)
        print(f"{name}: rel abs-max err = {err:.3e}")
        assert err < tol, f"{name} mismatch"
    print("SIM SELFTEST PASSED")


if __name__ == "__main__":
    _selftest_sim()


# revision 14
# speedup vs baseline: 1.1769x; 1.1769x over previous
"""Trainium2 Bass kernel for nn_AttentionUnit (attention pooling).

reference math:
    q = query @ Wq.T + bq;  k = keys @ Wk.T + bk
    score[b,l] = <k[b,l], q[b]>;  attn = softmax(score, axis=1)
    wsum[b] = sum_l attn[b,l] * keys[b,l];  return (wsum, attn)

Algebra: score[b,l] = <keys[b,l,:], qk[b,:]> + c[b], qk = (query@Wq.T+bq)@Wk;
c[b] is constant over l so it cancels in the softmax -> bk never matters and
the big keys projection disappears.  Memory-bound on reading keys once.

Per-batch-row work is mapped to the tensor engine as M=1 matmuls with the
keys tile as the N-moving operand (weights = a single qk / exp column, so
LDWEIGHTS is 1 column instead of 100+), packed 4-wide into the PE's 32-col
groups via tile_position so four rows compute concurrently:
  score_b = qk_b.T @ keysT_b      (K=i=128, N=L=200)  -> psum row 32g
  uws_b   = expT_b.T @ keysN_b    (K=l=100 x2 acc, N=D=128) -> psum row 32g
Rows land on psum partitions {0,32,64,96} x 8 round-offsets; ACT/DVE copies
with partition-stride-32 APs compact them to [b, .] tiles.  Softmax runs in
[b, l] layout: DVE reduce_max, ACT exp with bias=-max + accumulated sums,
DVE reciprocal/normalize.  exp is transposed (PE) and cast to fp16 for the
wsum pass; keysN is fp16 (keys are N(0,1): rel err ~1e-4, well in range).
"""

import sys

sys.path.insert(0, "/opt/trn_rl_repo")

import numpy as np

D = 128
L = 200
LH = L // 2  # 100
B_FULL = 8192
N_CORES = 8
B_CORE = B_FULL // N_CORES  # 1024

_NC_CACHE = {}


def build_nc(b_core=B_CORE, tile_b=16, kt_fp16=False):
    from contextlib import ExitStack

    import concourse.masks as masks
    import concourse.mybir as mybir
    import concourse.tile as tile
    from concourse.bacc import Bacc

    f32 = mybir.dt.float32
    f16 = mybir.dt.float16
    ktd = f16 if kt_fp16 else f32
    AF = mybir.ActivationFunctionType
    OP = mybir.AluOpType

    assert b_core % tile_b == 0 and tile_b % 4 == 0
    n_tiles = b_core // tile_b
    R = tile_b // 4  # rounds of 4 concurrent col-groups
    assert b_core % 128 == 0

    nc = Bacc()

    # i-major / l-major global layouts so each DMA descriptor is one long
    # contiguous run per partition
    keysT_d = nc.dram_tensor("keysT", [D, b_core, L], ktd, kind="ExternalInput")
    keysN_d = nc.dram_tensor("keysN", [LH, b_core, 2 * D], f16, kind="ExternalInput")
    queryT_d = nc.dram_tensor("queryT", [D, b_core], f32, kind="ExternalInput")
    wqT_d = nc.dram_tensor("wqT", [D, D], f32, kind="ExternalInput")
    wk_d = nc.dram_tensor("wk", [D, D], f32, kind="ExternalInput")
    bq_d = nc.dram_tensor("bq", [D, 1], f32, kind="ExternalInput")
    wsum_d = nc.dram_tensor("wsum", [b_core, D], f32, kind="ExternalOutput")
    attn_d = nc.dram_tensor("attn", [b_core, L], f32, kind="ExternalOutput")

    with tile.TileContext(nc) as tc, ExitStack() as ctx:
        const = ctx.enter_context(tc.tile_pool(name="const", bufs=1))
        identity = const.tile([128, 128], f32)
        masks.make_identity(nc, identity[:])
        queryT_s = const.tile([D, b_core], f32)
        nc.sync.dma_start(queryT_s[:], queryT_d[:])
        qkT_all = const.tile([D, b_core], ktd)

        kt_p = ctx.enter_context(tc.tile_pool(name="kt", bufs=2))
        kn_p = ctx.enter_context(tc.tile_pool(name="kn", bufs=4))
        sm_p = ctx.enter_context(tc.tile_pool(name="sm", bufs=3))
        out_p = ctx.enter_context(tc.tile_pool(name="outs", bufs=2))
        # PSUM: 4 + 2 + 2 = 8 banks
        ps_sc = ctx.enter_context(tc.tile_pool(name="ps_sc", bufs=1, space="PSUM"))
        ps_uw = ctx.enter_context(tc.tile_pool(name="ps_uw", bufs=1, space="PSUM"))
        ps_tr = ctx.enter_context(tc.tile_pool(name="ps_tr", bufs=2, space="PSUM"))
        # persistent psum blocks (memset once so the never-written garbage
        # rows between the M=1 outputs are initialized)
        sc_pss = []
        uws_pss = []
        for _k in range(2):
            _sp = ps_sc.tile([128, R * 256], f32, tag=f"sc{_k}")
            nc.vector.memset(_sp[:], 0.0)
            sc_pss.append(_sp)
            _up = ps_uw.tile([128, R * 128], f32, tag=f"uw{_k}")
            nc.vector.memset(_up[:], 0.0)
            uws_pss.append(_up)

        # ---------------- pre-pass: qkT = Wk-contract(WqT.T @ queryT + bq) --
        with tc.tile_pool(name="pre_sb", bufs=2) as pre_sb:
            wqT_s = pre_sb.tile([D, D], f32, tag="w")
            nc.sync.dma_start(wqT_s[:], wqT_d[:])
            wk_s = pre_sb.tile([D, D], f32, tag="w")
            nc.sync.dma_start(wk_s[:], wk_d[:])
            bq_s = pre_sb.tile([D, 1], f32, tag="b")
            nc.sync.dma_start(bq_s[:], bq_d[:])
            for t8 in range(b_core // 128):
                sl = slice(t8 * 128, (t8 + 1) * 128)
                qT_ps = ps_tr.tile([128, 128], f32, tag="tr")
                nc.tensor.matmul(qT_ps[:], wqT_s[:], queryT_s[:, sl])
                qT_sb = pre_sb.tile([128, 128], f32, tag="q")
                nc.scalar.activation(qT_sb[:], qT_ps[:], AF.Identity, bias=bq_s[:, 0:1])
                qk_ps = ps_tr.tile([128, 128], f32, tag="tr")
                nc.tensor.matmul(qk_ps[:], wk_s[:], qT_sb[:])
                nc.scalar.activation(qkT_all[:, sl], qk_ps[:], AF.Copy, bias=0.0)

        # ---------------- main pipeline -------------------------------------
        # working layout: local row j = 4*r + g lives at partition g, round r
        state = {}

        def emit_load_score(t):
            t0 = t * tile_b
            kT = kt_p.tile([D, tile_b, L], ktd, tag="kt")
            nc.sync.dma_start(kT[:], keysT_d[:, t0 : t0 + tile_b, :])
            kN = kn_p.tile([LH, tile_b, 2 * D], f16, tag="kn")
            nc.sync.dma_start(kN[:], keysN_d[:, t0 : t0 + tile_b, :])
            # score rows: group g -> psum partition 32g, round r -> 256-fl offset
            sc_ps = sc_pss[t % 2]
            for r in range(R):
                for g in range(4):
                    j = 4 * r + g
                    nc.tensor.matmul(
                        sc_ps[32 * g : 32 * g + 1, r * 256 : r * 256 + L],
                        qkT_all[:, t0 + j : t0 + j + 1],
                        kT[:, j, :],
                        tile_position=(0, 32 * g),
                    )
            state[t] = kN

        def emit_softmax(t):
            kN = state[t]
            sc_ps = sc_pss[t % 2]
            t0 = t * tile_b
            # full-width softmax over the sparse psum rows (only partitions
            # {0,32,64,96} carry data; other lanes compute garbage for free)
            scv = sc_ps[:].rearrange("p (r x) -> p r x", x=256)[:, :, 0:L]
            m128 = sm_p.tile([128, R], f32, tag="m")
            nc.vector.tensor_reduce(m128[:], scv, axis=mybir.AxisListType.X, op=OP.max)
            negm = sm_p.tile([128, R], f32, tag="nm")
            nc.vector.tensor_scalar_mul(negm[:], m128[:], -1.0)
            exp128 = sm_p.tile([128, R, L], f32, tag="e")
            s128 = sm_p.tile([128, R], f32, tag="s")
            for r in range(R):
                nc.scalar.activation(
                    exp128[:, r, :], scv[:, r, :], AF.Exp,
                    bias=negm[:, r : r + 1], accum_out=s128[:, r : r + 1],
                )
            r128 = sm_p.tile([128, R], f32, tag="r")
            nc.vector.reciprocal(r128[:], s128[:])
            attn128 = sm_p.tile([128, R, L], f32, tag="at")
            for r in range(R):
                nc.vector.tensor_scalar_mul(
                    attn128[:, r, :], exp128[:, r, :], r128[:, r : r + 1]
                )
            nc.scalar.dma_start(
                attn_d[t0 : t0 + tile_b, :].rearrange("(r g) l -> g r l", g=4),
                attn128[0:128:32, :, :],
            )
            # compact the valid exp rows via DMA, then transpose+cast to fp16
            exp4 = sm_p.tile([4, R, L], f32, tag="e4")
            nc.scalar.dma_start(exp4[:], exp128[0:128:32, :, :])
            etA = sm_p.tile([LH, tile_b], f16, tag="et16")
            etB = sm_p.tile([LH, tile_b], f16, tag="et16")
            for r in range(R):
                trA = ps_tr.tile([LH, 4], f32, tag="tr")
                nc.tensor.transpose(trA[:], exp4[:, r, 0:LH], identity[0:4, 0:4])
                nc.scalar.activation(etA[:, 4 * r : 4 * r + 4], trA[:], AF.Copy, bias=0.0)
                trB = ps_tr.tile([LH, 4], f32, tag="tr")
                nc.tensor.transpose(trB[:], exp4[:, r, LH:L], identity[0:4, 0:4])
                nc.scalar.activation(etB[:, 4 * r : 4 * r + 4], trB[:], AF.Copy, bias=0.0)
            state[t] = (kN, etA, etB, r128)

        def emit_back(t):
            t0 = t * tile_b
            kN, etA, etB, r128 = state.pop(t)
            uws_ps = uws_pss[t % 2]
            for r in range(R):
                for g in range(4):
                    j = 4 * r + g
                    out = uws_ps[32 * g : 32 * g + 1, r * 128 : (r + 1) * 128]
                    nc.tensor.matmul(
                        out, etA[:, j : j + 1], kN[:, j, 0:D],
                        start=True, stop=False, tile_position=(0, 32 * g),
                    )
                    nc.tensor.matmul(
                        out, etB[:, j : j + 1], kN[:, j, D : 2 * D],
                        start=False, stop=True, tile_position=(0, 32 * g),
                    )
            ws128 = out_p.tile([128, R, D], f32, tag="ws")
            uwv = uws_ps[:].rearrange("p (r x) -> p r x", x=D)
            r128b = r128[:].unsqueeze(2).broadcast_to([128, R, D])
            nc.vector.tensor_tensor(ws128[:], uwv, r128b, op=OP.mult)
            nc.scalar.dma_start(
                wsum_d[t0 : t0 + tile_b, :].rearrange("(r g) i -> g r i", g=4),
                ws128[0:128:32, :, :],
            )

        for t in range(n_tiles):
            emit_load_score(t)
            if t > 0:
                emit_softmax(t - 1)
            if t > 1:
                emit_back(t - 2)
        emit_softmax(n_tiles - 1)
        emit_back(n_tiles - 2)
        emit_back(n_tiles - 1)

    nc.finalize()
    return nc


def get_nc(**kw):
    key = tuple(sorted(kw.items()))
    if key not in _NC_CACHE:
        _NC_CACHE[key] = build_nc(**kw)
    return _NC_CACHE[key]


def _prep_in_maps(query, keys, Wq, bq, Wk, b_core=B_CORE, kt_fp16=False):
    query = np.asarray(query, dtype=np.float32)
    keys = np.asarray(keys, dtype=np.float32)
    B = query.shape[0]
    ktd = np.float16 if kt_fp16 else np.float32
    keysT = keys.transpose(2, 0, 1).astype(ktd)  # [D, B, L] i-major
    # pair rows l and l+100 (512B contiguous fp16 rows), l-major globally
    keysN = (
        keys.reshape(B, 2, LH, D).transpose(2, 0, 1, 3).reshape(LH, B, 2 * D)
    ).astype(np.float16)
    wqT = np.ascontiguousarray(np.asarray(Wq, dtype=np.float32).T)
    wk = np.ascontiguousarray(np.asarray(Wk, dtype=np.float32))
    bqc = np.ascontiguousarray(np.asarray(bq, dtype=np.float32).reshape(D, 1))
    queryT = np.ascontiguousarray(query.T)
    in_maps = []
    for c in range(B // b_core):
        s = slice(c * b_core, (c + 1) * b_core)
        in_maps.append(
            {
                "keysT": np.ascontiguousarray(keysT[:, s, :]),
                "keysN": np.ascontiguousarray(keysN[:, s, :]),
                "queryT": np.ascontiguousarray(queryT[:, s]),
                "wqT": wqT,
                "wk": wk,
                "bq": bqc,
            }
        )
    return in_maps


def kernel(query, keys, Wq, bq, Wk, bk):
    from concourse.bass_utils import run_bass_kernel_spmd

    nc = get_nc()
    in_maps = _prep_in_maps(query, keys, Wq, bq, Wk)
    res = run_bass_kernel_spmd(nc, in_maps, list(range(N_CORES))).results
    wsum = np.concatenate([r["wsum"] for r in res], axis=0)
    attn = np.concatenate([r["attn"] for r in res], axis=0)
    return (wsum, attn)


# ----------------------------------------------------------------------------
def _np_reference(query, keys, Wq, bq, Wk, bk):
    q = query @ Wq.T + bq
    k = keys @ Wk.T + bk
    score = np.einsum("bld,bd->bl", k, q)
    m = score.max(axis=1, keepdims=True)
    e = np.exp(score - m)
    attn = e / e.sum(axis=1, keepdims=True)
    ws = np.einsum("bl,bld->bd", attn, keys)
    return ws, attn


def _make_inputs(b, seed=0):
    rng = np.random.default_rng(seed)
    s = 1.0 / np.sqrt(D)
    return dict(
        query=rng.standard_normal((b, D), dtype=np.float32),
        keys=rng.standard_normal((b, L, D), dtype=np.float32),
        Wq=rng.standard_normal((D, D), dtype=np.float32) * s,
        bq=rng.standard_normal((D,), dtype=np.float32) * s,
        Wk=rng.standard_normal((D, D), dtype=np.float32) * s,
        bk=rng.standard_normal((D,), dtype=np.float32) * s,
    )


def _selftest_sim(b_core=128, tile_b=16, kt_fp16=False):
    from concourse.bass_interp import CoreSim

    ins = _make_inputs(b_core)
    nc = build_nc(b_core=b_core, tile_b=tile_b, kt_fp16=kt_fp16)
    in_map = _prep_in_maps(
        ins["query"], ins["keys"], ins["Wq"], ins["bq"], ins["Wk"],
        b_core=b_core, kt_fp16=kt_fp16,
    )[0]
    sim = CoreSim(nc)
    for name, arr in in_map.items():
        sim.tensor(name)[:] = arr
    sim.simulate()
    wsum = np.array(sim.tensor("wsum"))
    attn = np.array(sim.tensor("attn"))
    ws_ref, attn_ref = _np_reference(
        ins["query"], ins["keys"], ins["Wq"], ins["bq"], ins["Wk"], ins["bk"]
    )
    for name, got, ref, tol in (
        ("wsum", wsum, ws_ref, 2e-3),
        ("attn", attn, attn_ref, 2e-2 if kt_fp16 else 2e-4),
    ):
        err = np.abs(got - ref).max() / (np.abs(ref).max() + 1e-30)
        print(f"{name}: rel abs-max err = {err:.3e}")
        assert err < tol, f"{name} mismatch"
    print("SIM SELFTEST PASSED")


if __name__ == "__main__":
    _selftest_sim()


# revision 15
# speedup vs baseline: 1.3142x; 1.1167x over previous
"""Trainium2 Bass kernel for nn_AttentionUnit (attention pooling).

reference math:
    q = query @ Wq.T + bq;  k = keys @ Wk.T + bk
    score[b,l] = <k[b,l], q[b]>;  attn = softmax(score, axis=1)
    wsum[b] = sum_l attn[b,l] * keys[b,l];  return (wsum, attn)

Algebra: score[b,l] = <keys[b,l,:], qk[b,:]> + c[b], qk = (query@Wq.T+bq)@Wk;
c[b] is constant over l so it cancels in the softmax -> bk never matters and
the big keys projection disappears.  Memory-bound on reading keys once.

Per-batch-row work is mapped to the tensor engine as M=1 matmuls with the
keys tile as the N-moving operand (weights = a single qk / exp column, so
LDWEIGHTS is 1 column instead of 100+), packed 4-wide into the PE's 32-col
groups via tile_position so four rows compute concurrently:
  score_b = qk_b.T @ keysT_b      (K=i=128, N=L=200)  -> psum row 32g
  uws_b   = expT_b.T @ keysN_b    (K=l=100 x2 acc, N=D=128) -> psum row 32g
Rows land on psum partitions {0,32,64,96} x 8 round-offsets; ACT/DVE copies
with partition-stride-32 APs compact them to [b, .] tiles.  Softmax runs in
[b, l] layout: DVE reduce_max, ACT exp with bias=-max + accumulated sums,
DVE reciprocal/normalize.  exp is transposed (PE) and cast to fp16 for the
wsum pass; keysN is fp16 (keys are N(0,1): rel err ~1e-4, well in range).
"""

import sys

sys.path.insert(0, "/opt/trn_rl_repo")

import numpy as np

D = 128
L = 200
LH = L // 2  # 100
B_FULL = 8192
N_CORES = 8
B_CORE = B_FULL // N_CORES  # 1024

_NC_CACHE = {}


def build_nc(b_core=B_CORE, tile_b=16, kt_fp16=False):
    from contextlib import ExitStack

    import concourse.masks as masks
    import concourse.mybir as mybir
    import concourse.tile as tile
    from concourse.bacc import Bacc

    f32 = mybir.dt.float32
    f16 = mybir.dt.float16
    ktd = f16 if kt_fp16 else f32
    AF = mybir.ActivationFunctionType
    OP = mybir.AluOpType

    assert b_core % tile_b == 0 and tile_b % 4 == 0
    n_tiles = b_core // tile_b
    R = tile_b // 4  # rounds of 4 concurrent col-groups
    assert b_core % 128 == 0

    nc = Bacc()

    # i-major / l-major global layouts so each DMA descriptor is one long
    # contiguous run per partition
    keysT_d = nc.dram_tensor("keysT", [D, b_core, L], ktd, kind="ExternalInput")
    keysN_d = nc.dram_tensor("keysN", [LH, b_core, 2 * D], f16, kind="ExternalInput")
    queryT_d = nc.dram_tensor("queryT", [D, b_core], f32, kind="ExternalInput")
    wqT_d = nc.dram_tensor("wqT", [D, D], f32, kind="ExternalInput")
    wk_d = nc.dram_tensor("wk", [D, D], f32, kind="ExternalInput")
    bq_d = nc.dram_tensor("bq", [D, 1], f32, kind="ExternalInput")
    wsum_d = nc.dram_tensor("wsum", [b_core, D], f32, kind="ExternalOutput")
    attn_d = nc.dram_tensor("attn", [b_core, L], f32, kind="ExternalOutput")

    with tile.TileContext(nc) as tc, ExitStack() as ctx:
        const = ctx.enter_context(tc.tile_pool(name="const", bufs=1))
        identity = const.tile([128, 128], f32)
        masks.make_identity(nc, identity[:])
        queryT_s = const.tile([D, b_core], f32)
        nc.sync.dma_start(queryT_s[:], queryT_d[:])
        qkT_all = const.tile([D, b_core], ktd)

        kt_p = ctx.enter_context(tc.tile_pool(name="kt", bufs=2))
        kn_p = ctx.enter_context(tc.tile_pool(name="kn", bufs=4))
        sm_p = ctx.enter_context(tc.tile_pool(name="sm", bufs=3))
        out_p = ctx.enter_context(tc.tile_pool(name="outs", bufs=2))
        # PSUM: 4 + 2 + 2 = 8 banks
        ps_sc = ctx.enter_context(tc.tile_pool(name="ps_sc", bufs=1, space="PSUM"))
        ps_uw = ctx.enter_context(tc.tile_pool(name="ps_uw", bufs=1, space="PSUM"))
        ps_tr = ctx.enter_context(tc.tile_pool(name="ps_tr", bufs=2, space="PSUM"))
        # persistent psum blocks (memset once so the never-written garbage
        # rows between the M=1 outputs are initialized)
        sc_pss = []
        uws_pss = []
        for _k in range(2):
            _sp = ps_sc.tile([128, R * 256], f32, tag=f"sc{_k}")
            nc.vector.memset(_sp[:], 0.0)
            sc_pss.append(_sp)
            _up = ps_uw.tile([128, R * 128], f32, tag=f"uw{_k}")
            nc.vector.memset(_up[:], 0.0)
            uws_pss.append(_up)

        # ---------------- pre-pass: qkT = Wk-contract(WqT.T @ queryT + bq) --
        with tc.tile_pool(name="pre_sb", bufs=2) as pre_sb:
            wqT_s = pre_sb.tile([D, D], f32, tag="w")
            nc.sync.dma_start(wqT_s[:], wqT_d[:])
            wk_s = pre_sb.tile([D, D], f32, tag="w")
            nc.sync.dma_start(wk_s[:], wk_d[:])
            bq_s = pre_sb.tile([D, 1], f32, tag="b")
            nc.sync.dma_start(bq_s[:], bq_d[:])
            for t8 in range(b_core // 128):
                sl = slice(t8 * 128, (t8 + 1) * 128)
                qT_ps = ps_tr.tile([128, 128], f32, tag="tr")
                nc.tensor.matmul(qT_ps[:], wqT_s[:], queryT_s[:, sl])
                qT_sb = pre_sb.tile([128, 128], f32, tag="q")
                nc.scalar.activation(qT_sb[:], qT_ps[:], AF.Identity, bias=bq_s[:, 0:1])
                qk_ps = ps_tr.tile([128, 128], f32, tag="tr")
                nc.tensor.matmul(qk_ps[:], wk_s[:], qT_sb[:])
                nc.scalar.activation(qkT_all[:, sl], qk_ps[:], AF.Copy, bias=0.0)

        # ---------------- main pipeline -------------------------------------
        # working layout: local row j = 4*r + g lives at partition g, round r
        state = {}

        def emit_load_score(t):
            t0 = t * tile_b
            kT = kt_p.tile([D, tile_b, L], ktd, tag="kt")
            nc.sync.dma_start(kT[:], keysT_d[:, t0 : t0 + tile_b, :])
            kN = kn_p.tile([LH, tile_b, 2 * D], f16, tag="kn")
            nc.sync.dma_start(kN[:], keysN_d[:, t0 : t0 + tile_b, :])
            # score rows: group g -> psum partition 32g, round r -> 256-fl offset
            sc_ps = sc_pss[t % 2]
            for r in range(R):
                for g in range(4):
                    j = 4 * r + g
                    nc.tensor.matmul(
                        sc_ps[32 * g : 32 * g + 1, r * 256 : r * 256 + L],
                        qkT_all[:, t0 + j : t0 + j + 1],
                        kT[:, j, :],
                        tile_position=(0, 32 * g),
                    )
            state[t] = kN

        def emit_softmax(t):
            kN = state[t]
            sc_ps = sc_pss[t % 2]
            t0 = t * tile_b
            # full-width softmax over the sparse psum rows (only partitions
            # {0,32,64,96} carry data; other lanes compute garbage for free)
            scv = sc_ps[:].rearrange("p (r x) -> p r x", x=256)[:, :, 0:L]
            m128 = sm_p.tile([128, R], f32, tag="m")
            nc.vector.tensor_reduce(m128[:], scv, axis=mybir.AxisListType.X, op=OP.max)
            negm = sm_p.tile([128, R], f32, tag="nm")
            nc.vector.tensor_scalar_mul(negm[:], m128[:], -1.0)
            exp128 = sm_p.tile([128, R, L], f32, tag="e")
            s128 = sm_p.tile([128, R], f32, tag="s")
            for r in range(R):
                nc.scalar.activation(
                    exp128[:, r, :], scv[:, r, :], AF.Exp,
                    bias=negm[:, r : r + 1], accum_out=s128[:, r : r + 1],
                )
            r128 = sm_p.tile([128, R], f32, tag="r")
            nc.vector.reciprocal(r128[:], s128[:])
            attn128 = sm_p.tile([128, R, L], f32, tag="at")
            for r in range(R):
                nc.vector.tensor_scalar_mul(
                    attn128[:, r, :], exp128[:, r, :], r128[:, r : r + 1]
                )
            nc.sync.dma_start(
                attn_d[t0 : t0 + tile_b, :].rearrange("(r g) l -> g r l", g=4),
                attn128[0:128:32, :, :],
            )
            # compact the valid exp rows via DMA, then transpose+cast to fp16
            exp4 = sm_p.tile([4, R, L], f32, tag="e4")
            nc.sync.dma_start(exp4[:], exp128[0:128:32, :, :])
            etA = sm_p.tile([LH, tile_b], f16, tag="et16")
            etB = sm_p.tile([LH, tile_b], f16, tag="et16")
            for r in range(R):
                trA = ps_tr.tile([LH, 4], f32, tag="tr")
                nc.tensor.transpose(trA[:], exp4[:, r, 0:LH], identity[0:4, 0:4])
                nc.vector.tensor_copy(etA[:, 4 * r : 4 * r + 4], trA[:])
                trB = ps_tr.tile([LH, 4], f32, tag="tr")
                nc.tensor.transpose(trB[:], exp4[:, r, LH:L], identity[0:4, 0:4])
                nc.vector.tensor_copy(etB[:, 4 * r : 4 * r + 4], trB[:])
            state[t] = (kN, etA, etB, r128)

        def emit_back(t):
            t0 = t * tile_b
            kN, etA, etB, r128 = state.pop(t)
            uws_ps = uws_pss[t % 2]
            for r in range(R):
                for g in range(4):
                    j = 4 * r + g
                    out = uws_ps[32 * g : 32 * g + 1, r * 128 : (r + 1) * 128]
                    nc.tensor.matmul(
                        out, etA[:, j : j + 1], kN[:, j, 0:D],
                        start=True, stop=False, tile_position=(0, 32 * g),
                    )
                    nc.tensor.matmul(
                        out, etB[:, j : j + 1], kN[:, j, D : 2 * D],
                        start=False, stop=True, tile_position=(0, 32 * g),
                    )
            ws128 = out_p.tile([128, R, D], f32, tag="ws")
            uwv = uws_ps[:].rearrange("p (r x) -> p r x", x=D)
            r128b = r128[:].unsqueeze(2).broadcast_to([128, R, D])
            nc.vector.tensor_tensor(ws128[:], uwv, r128b, op=OP.mult)
            nc.sync.dma_start(
                wsum_d[t0 : t0 + tile_b, :].rearrange("(r g) i -> g r i", g=4),
                ws128[0:128:32, :, :],
            )

        for t in range(n_tiles):
            emit_load_score(t)
            if t > 0:
                emit_softmax(t - 1)
            if t > 1:
                emit_back(t - 2)
        emit_softmax(n_tiles - 1)
        emit_back(n_tiles - 2)
        emit_back(n_tiles - 1)

    nc.finalize()
    return nc


def get_nc(**kw):
    key = tuple(sorted(kw.items()))
    if key not in _NC_CACHE:
        _NC_CACHE[key] = build_nc(**kw)
    return _NC_CACHE[key]


def _prep_in_maps(query, keys, Wq, bq, Wk, b_core=B_CORE, kt_fp16=False):
    query = np.asarray(query, dtype=np.float32)
    keys = np.asarray(keys, dtype=np.float32)
    B = query.shape[0]
    ktd = np.float16 if kt_fp16 else np.float32
    keysT = keys.transpose(2, 0, 1).astype(ktd)  # [D, B, L] i-major
    # pair rows l and l+100 (512B contiguous fp16 rows), l-major globally
    keysN = (
        keys.reshape(B, 2, LH, D).transpose(2, 0, 1, 3).reshape(LH, B, 2 * D)
    ).astype(np.float16)
    wqT = np.ascontiguousarray(np.asarray(Wq, dtype=np.float32).T)
    wk = np.ascontiguousarray(np.asarray(Wk, dtype=np.float32))
    bqc = np.ascontiguousarray(np.asarray(bq, dtype=np.float32).reshape(D, 1))
    queryT = np.ascontiguousarray(query.T)
    in_maps = []
    for c in range(B // b_core):
        s = slice(c * b_core, (c + 1) * b_core)
        in_maps.append(
            {
                "keysT": np.ascontiguousarray(keysT[:, s, :]),
                "keysN": np.ascontiguousarray(keysN[:, s, :]),
                "queryT": np.ascontiguousarray(queryT[:, s]),
                "wqT": wqT,
                "wk": wk,
                "bq": bqc,
            }
        )
    return in_maps


def kernel(query, keys, Wq, bq, Wk, bk):
    from concourse.bass_utils import run_bass_kernel_spmd

    nc = get_nc()
    in_maps = _prep_in_maps(query, keys, Wq, bq, Wk)
    res = run_bass_kernel_spmd(nc, in_maps, list(range(N_CORES))).results
    wsum = np.concatenate([r["wsum"] for r in res], axis=0)
    attn = np.concatenate([r["attn"] for r in res], axis=0)
    return (wsum, attn)


# ----------------------------------------------------------------------------
def _np_reference(query, keys, Wq, bq, Wk, bk):
    q = query @ Wq.T + bq
    k = keys @ Wk.T + bk
    score = np.einsum("bld,bd->bl", k, q)
    m = score.max(axis=1, keepdims=True)
    e = np.exp(score - m)
    attn = e / e.sum(axis=1, keepdims=True)
    ws = np.einsum("bl,bld->bd", attn, keys)
    return ws, attn


def _make_inputs(b, seed=0):
    rng = np.random.default_rng(seed)
    s = 1.0 / np.sqrt(D)
    return dict(
        query=rng.standard_normal((b, D), dtype=np.float32),
        keys=rng.standard_normal((b, L, D), dtype=np.float32),
        Wq=rng.standard_normal((D, D), dtype=np.float32) * s,
        bq=rng.standard_normal((D,), dtype=np.float32) * s,
        Wk=rng.standard_normal((D, D), dtype=np.float32) * s,
        bk=rng.standard_normal((D,), dtype=np.float32) * s,
    )


def _selftest_sim(b_core=128, tile_b=16, kt_fp16=False):
    from concourse.bass_interp import CoreSim

    ins = _make_inputs(b_core)
    nc = build_nc(b_core=b_core, tile_b=tile_b, kt_fp16=kt_fp16)
    in_map = _prep_in_maps(
        ins["query"], ins["keys"], ins["Wq"], ins["bq"], ins["Wk"],
        b_core=b_core, kt_fp16=kt_fp16,
    )[0]
    sim = CoreSim(nc)
    for name, arr in in_map.items():
        sim.tensor(name)[:] = arr
    sim.simulate()
    wsum = np.array(sim.tensor("wsum"))
    attn = np.array(sim.tensor("attn"))
    ws_ref, attn_ref = _np_reference(
        ins["query"], ins["keys"], ins["Wq"], ins["bq"], ins["Wk"], ins["bk"]
    )
    for name, got, ref, tol in (
        ("wsum", wsum, ws_ref, 2e-3),
        ("attn", attn, attn_ref, 2e-2 if kt_fp16 else 2e-4),
    ):
        err = np.abs(got - ref).max() / (np.abs(ref).max() + 1e-30)
        print(f"{name}: rel abs-max err = {err:.3e}")
        assert err < tol, f"{name} mismatch"
    print("SIM SELFTEST PASSED")


if __name__ == "__main__":
    _selftest_sim()
